# revision 3
# baseline (speedup 1.0000x reference)
"""DeepSeekMoE kernel for 8x Trainium2 NeuronCores.

Strategy (expert-parallel, host dispatch):
  - Host computes the (tiny) sigmoid gate + top-2 routing in fp32 numpy and
    gathers each expert's tokens (capacity = max expert count, rounded to 128).
  - Core e runs expert e's FFN on its gathered tokens plus the shared-expert
    FFN on the e-th block of 512 tokens. All matmuls in bf16 (fp32 PSUM
    accumulation), weights/activations laid out features-on-partitions so no
    transposes are needed on device.
  - Host applies combine weights and scatter-adds expert outputs (pure
    gather/FMA, no atomics needed) and adds the shared outputs.
"""

import sys
import types

sys.path.insert(0, "/opt/trn_rl_repo")

import numpy as np
import ml_dtypes

import concourse.bass as bass
import concourse.mybir as mybir
import concourse.tile as tile
from concourse.bass_utils import run_bass_kernel_spmd

# Problem constants (nn_DeepSeekMoE_91336774516862)
B, V, L, H, E = 4, 8, 128, 1024, 8
F = 4 * H
T = B * V * L          # 4096 tokens
TOP_K = 2
N_CORES = 8
S = T // N_CORES       # shared-expert tokens per core
KH = H // 128          # 8 k-tiles over H
KF = F // 128          # 32 k-tiles over F
NCH = 512              # token chunk (one PSUM bank of fp32)

BF16 = mybir.dt.bfloat16
F32 = mybir.dt.float32

# ---------------------------------------------------------------------------
# Patch: tile's kernel-tail drain aggregates one wait per logical proc onto a
# single InstDrain, but TPB_CTRL supports exactly 1 wait in this walrus
# ("Too many sync wait commands"). Split into one drain per wait.
import bass_rust
from concourse.vector_clock import ScopedClock


def _patched_drain_and_barrier(self, tick_clock, wait_clock):
    nc = self.nc
    drain_inst = nc.sync.drain()
    wait_clock.add_sem_waits(
        drain_inst.ins, ScopedClock({None: tick_clock.global_clock})
    )
    si = drain_inst.ins.sync_info
    waits = list(si.on_wait or []) if si is not None else []
    upds = list(si.on_update or []) if si is not None else []
    if len(waits) > 1:
        drain_inst.ins.sync_info = bass_rust.SyncInfo(
            on_wait=[waits[0]], on_update=upds
        )
        for w in waits[1:]:
            extra = nc.sync.drain()
            extra.ins.sync_info = bass_rust.SyncInfo(on_wait=[w], on_update=[])
    nc.all_engine_barrier()
    assert self.sems is not None
    popped = nc._tile_sem_poison_stack.pop()
    assert popped is self._sem_poison
    nc.clear_and_free_semaphores(list(self.sems.allocated().values()))
    nc.all_engine_barrier()


tile.TileContext._drain_and_barrier = _patched_drain_and_barrier


def _normalize_waits(nc, max_waits=1):
    """Walrus in this container accepts at most one sync-wait per instruction;
    hoist extras onto injected same-engine nops placed just before."""
    n_fix = 0
    for f in nc.m.functions:
        for b in f.blocks:
            insts = b.instructions
            out = []
            for ins in insts:
                si = ins.sync_info
                waits = list(si.on_wait) if si is not None and si.on_wait else []
                if len(waits) > max_waits:
                    upds = list(si.on_update) if si.on_update else []
                    keep = waits[:max_waits]
                    for w in waits[max_waits:]:
                        nop = mybir.InstNoOp(
                            name=f"{ins.name}_waitsplit{n_fix}",
                            engine=ins.engine,
                            bass_nofuse=True,
                            sync_info=mybir.SyncInfo(on_wait=[w], on_update=[]),
                        )
                        out.append(nop)
                        n_fix += 1
                    ins.sync_info = mybir.SyncInfo(on_wait=keep, on_update=upds)
                out.append(ins)
            if len(out) != len(insts):
                b.instructions = out
    return n_fix


# ---------------------------------------------------------------------------
def _ffn_segment(nc, tc, pools, seg, xt_dram, w1_dram, w2_dram, b1_dram, b2_dram,
                 out_dram, Ct):
    """y = gelu(x @ w1.T + b1) @ w2.T + b2, features-on-partitions.

    xt_dram: [H, Ct] bf16 (tokens transposed)   w1_dram: [KF][H, 128] bf16
    w2_dram: [KH][F, 128] bf16                  out_dram: [H, Ct] f32
    """
    const, xt_pool, w_pool, h_pool, out_pool, ps1_pool, ps2_pool = pools

    b1t = const.tile([128, KF], F32, tag=f"b1_{seg}")
    nc.sync.dma_start(b1t[:], b1_dram.rearrange("(m p) -> p m", p=128))
    b2t = const.tile([128, KH], F32, tag=f"b2_{seg}")
    nc.sync.dma_start(b2t[:], b2_dram.rearrange("(m p) -> p m", p=128))

    xt = []
    for kh in range(KH):
        t = xt_pool.tile([128, Ct], BF16, tag=f"xt_{kh}")
        nc.sync.dma_start(t[:], xt_dram[kh * 128:(kh + 1) * 128, :])
        xt.append(t)

    w1t = []
    for mf in range(KF):
        t = w_pool.tile([128, KH, 128], BF16, tag=f"w1_{mf}")
        nc.sync.dma_start(t[:], w1_dram[mf].rearrange("(k p) f -> p k f", p=128))
        w1t.append(t)
    w2t = []
    for mh in range(KH):
        t = w_pool.tile([128, KF, 128], BF16, tag=f"w2_{mh}")
        nc.sync.dma_start(t[:], w2_dram[mh].rearrange("(k p) f -> p k f", p=128))
        w2t.append(t)

    for c0 in range(0, Ct, NCH):
        nch = min(NCH, Ct - c0)
        h = h_pool.tile([128, KF, nch], BF16, tag="h")
        for mf in range(KF):
            ps1 = ps1_pool.tile([128, nch], F32, tag="ps1")
            for kh in range(KH):
                nc.tensor.matmul(
                    ps1[:],
                    w1t[mf][:, kh, :],
                    xt[kh][:, c0:c0 + nch],
                    start=(kh == 0),
                    stop=(kh == KH - 1),
                )
            nc.scalar.activation(
                h[:, mf, :], ps1[:],
                mybir.ActivationFunctionType.Gelu,
                bias=b1t[:, mf:mf + 1],
            )
        for mh in range(KH):
            ps2 = ps2_pool.tile([128, nch], F32, tag="ps2")
            for kf in range(KF):
                nc.tensor.matmul(
                    ps2[:],
                    w2t[mh][:, kf, :],
                    h[:, kf, :],
                    start=(kf == 0),
                    stop=(kf == KF - 1),
                )
            oc = out_pool.tile([128, nch], F32, tag="oc")
            nc.scalar.activation(
                oc[:], ps2[:],
                mybir.ActivationFunctionType.Identity,
                bias=b2t[:, mh:mh + 1],
            )
            nc.sync.dma_start(out_dram[mh * 128:(mh + 1) * 128, c0:c0 + nch], oc[:])


def build_nc(C):
    nc = bass.Bass()
    xeT = nc.dram_tensor("xeT", [H, C], BF16, kind="ExternalInput")
    xsT = nc.dram_tensor("xsT", [H, S], BF16, kind="ExternalInput")
    w1c = nc.dram_tensor("w1c", [KF, H, 128], BF16, kind="ExternalInput")
    w2c = nc.dram_tensor("w2c", [KH, F, 128], BF16, kind="ExternalInput")
    s1c = nc.dram_tensor("s1c", [KF, H, 128], BF16, kind="ExternalInput")
    s2c = nc.dram_tensor("s2c", [KH, F, 128], BF16, kind="ExternalInput")
    b1 = nc.dram_tensor("b1", [F], F32, kind="ExternalInput")
    b2 = nc.dram_tensor("b2", [H], F32, kind="ExternalInput")
    sb1 = nc.dram_tensor("sb1", [F], F32, kind="ExternalInput")
    sb2 = nc.dram_tensor("sb2", [H], F32, kind="ExternalInput")
    yT = nc.dram_tensor("yT", [H, C], F32, kind="ExternalOutput")
    ysT = nc.dram_tensor("ysT", [H, S], F32, kind="ExternalOutput")

    with tile.TileContext(nc) as tc:
        with (
            tc.tile_pool(name="const", bufs=1) as const,
            tc.tile_pool(name="xt", bufs=1) as xt_pool,
            tc.tile_pool(name="w", bufs=1) as w_pool,
            tc.tile_pool(name="h", bufs=1) as h_pool,
            tc.tile_pool(name="out", bufs=4) as out_pool,
            tc.tile_pool(name="ps1", bufs=3, space="PSUM") as ps1_pool,
            tc.tile_pool(name="ps2", bufs=3, space="PSUM") as ps2_pool,
        ):
            pools = (const, xt_pool, w_pool, h_pool, out_pool, ps1_pool, ps2_pool)
            _ffn_segment(nc, tc, pools, 0, xeT, w1c, w2c, b1, b2, yT, C)
            _ffn_segment(nc, tc, pools, 1, xsT, s1c, s2c, sb1, sb2, ysT, S)
    nc.finalize()
    _normalize_waits(nc)
    return nc


_NC_CACHE = {}


def _get_nc(C):
    if C not in _NC_CACHE:
        _NC_CACHE[C] = build_nc(C)
    return _NC_CACHE[C]


def _w1_chunks(w1):
    # w1 [F, H] -> [KF, H, 128] where [mf] = w1[mf*128:(mf+1)*128, :].T
    return np.ascontiguousarray(
        np.transpose(w1.reshape(KF, 128, H), (0, 2, 1))
    ).astype(ml_dtypes.bfloat16)


def _w2_chunks(w2):
    # w2 [H, F] -> [KH, F, 128] where [mh] = w2[mh*128:(mh+1)*128, :].T
    return np.ascontiguousarray(
        np.transpose(w2.reshape(KH, 128, F), (0, 2, 1))
    ).astype(ml_dtypes.bfloat16)


def prepare(x, gate_w, gate_b, bias, sh_w1, sh_b1, sh_w2, sh_b2,
            ex_w1, ex_b1, ex_w2, ex_b2):
    """Host routing + per-core input maps. Returns (nc, in_maps, meta)."""
    x_flat = np.ascontiguousarray(x.reshape(T, H))

    # fp32 sigmoid gate + top-2 (stable argsort matches jax.lax.top_k ties)
    logits = x_flat @ gate_w.T + (gate_b + bias)
    scores = 1.0 / (1.0 + np.exp(-logits))
    order = np.argsort(-scores, axis=1, kind="stable")
    top_idx = order[:, :TOP_K]                      # [T, 2]
    top_w = np.take_along_axis(scores, top_idx, axis=1)

    idx_e, w_e = [], []
    for e in range(E):
        m = top_idx == e                            # [T, 2]
        sel = np.nonzero(m.any(axis=1))[0]
        idx_e.append(sel)
        w_e.append(np.where(m[sel, 0], top_w[sel, 0], top_w[sel, 1]))
    counts = np.array([len(i) for i in idx_e])
    C = max(128, int(-(-counts.max() // 128) * 128))

    nc = _get_nc(C)

    sh1c, sh2c = _w1_chunks(sh_w1), _w2_chunks(sh_w2)
    sb1 = np.ascontiguousarray(sh_b1, dtype=np.float32)
    sb2 = np.ascontiguousarray(sh_b2, dtype=np.float32)

    in_maps = []
    for e in range(E):
        xe = np.zeros((C, H), dtype=np.float32)
        xe[: counts[e]] = x_flat[idx_e[e]]
        xeT = np.ascontiguousarray(xe.T).astype(ml_dtypes.bfloat16)
        xs = x_flat[e * S:(e + 1) * S]
        xsT = np.ascontiguousarray(xs.T).astype(ml_dtypes.bfloat16)
        in_maps.append({
            "xeT": xeT,
            "xsT": xsT,
            "w1c": _w1_chunks(ex_w1[e]),
            "w2c": _w2_chunks(ex_w2[e]),
            "s1c": sh1c,
            "s2c": sh2c,
            "b1": np.ascontiguousarray(ex_b1[e], dtype=np.float32),
            "b2": np.ascontiguousarray(ex_b2[e], dtype=np.float32),
            "sb1": sb1,
            "sb2": sb2,
        })
    meta = (idx_e, w_e, counts, C)
    return nc, in_maps, meta


def combine(results, meta, out_shape):
    idx_e, w_e, counts, C = meta
    out = np.zeros((T, H), dtype=np.float32)
    for e in range(E):
        ysT = results[e]["ysT"]                      # [H, S]
        out[e * S:(e + 1) * S] += ysT.T
        if counts[e]:
            ye = results[e]["yT"][:, : counts[e]].T  # [counts, H]
            out[idx_e[e]] += w_e[e][:, None] * ye
    return out.reshape(out_shape)


def kernel(**inputs):
    inputs = {k: np.asarray(v) for k, v in inputs.items()}
    out_shape = inputs["x"].shape
    nc, in_maps, meta = prepare(**inputs)
    res = run_bass_kernel_spmd(
        nc, in_maps, core_ids=list(range(N_CORES)), trace=False
    )
    return combine(res.results, meta, out_shape)
